# revision 72
# baseline (speedup 1.0000x reference)
"""Multi-head attention (B=4, S=2048, D=1024, H=16) on 8 TRN2 NeuronCores.

Sharding: DP=4 over batch x TP=2 over heads. Core c handles batch c//2 and
heads 8*(c%2) .. 8*(c%2)+8. Each core computes a partial output [S, D] (its
heads' contribution to the out-projection); the host sums the two TP partials
per batch and adds the output bias.

Key compaction: the key-padding mask removes ~half the keys, so the host
gathers unmasked key rows per batch (padded to a multiple of 128). k/v
projections and attention only touch NKV ~= S/2 keys; padding keys carry a
big negative additive bias fused into the exp so they contribute exactly 0.

On-chip layouts (all matmul operands bf16, accumulation fp32 in PSUM):
  qT/kT : [hd, seq] with the two heads of a pair stacked on partitions
          (0-63 / 64-127).  The two K=64 score matmuls of a pair are
          row-group-packed but the PE never overlaps their streams; a
          full-array matmul following a pair pays ~107ns of drain.  qT is
          PRE-SCALED by ALPHA = SCALE * 128/ln2 so PSUM scores are in
          "bf16-bits" units.
  exp   : split across two engines so neither paces the kernel. ScalarE
          computes exact exp via its free affine (scale=1/A, bias=mask);
          for kc in DVE_KCS (late in the step: the previous step's norm
          chain occupies VectorE during kc0-3) VectorE computes a
          Schraudolph exp in ONE op: bits16 = max(scores + B, 0) converted
          to int16 and bit-viewed as bf16 (linear-in-log2 approx, ~1.8% rms
          sawtooth, softmax-scale invariant; masked keys -> exactly 0).
  v_aug : [keys, v | ones(64)] -> the ctx matmul accumulates ctxT (rows 0-63)
          and the softmax denominator replicated across rows 64-127, so the
          normalization is a copy + fast-reciprocal + multiply on VectorE
          (fast-reciprocal breaks on shifted/PSUM APs -> the copy stays).

Scheduling: the TileScheduler re-orders instructions by readiness (emission
order is only a hint), so per-engine load balance and dependency slack
matter more than emission adjacency.  The exp-paced kc loop leaves PE slack
that is filled by a global queue of deferred matmul chunks (remaining
projections, v units, next q window, previous step's out-projection);
force() emits any still-queued unit a step depends on before its consumers.
v units are queued late (their wv DMA lands late; early placement
head-of-line-blocks the PE queue).  Input DMAs are spread over the Sync,
GpSimd and Scalar issue queues (a dma_start costs ~0.6us of issue time on
its queue; ScalarE's queue is kept light since it dispatches most exps).
Prologue: gating DMAs are kc-interleaved so the first projection matmuls
gate on ~0.3MB, and garbage micro-matmuls warm the PE clock gate (HAM)
during the DMA window.  Drain: the last out-projections are phase-split
(khp0,1 / khp2 / khp3+copy+DMA) so the PE bridges the final norm chains,
and the final copies/DMAs are split across ScalarE+VectorE and two queues.

Measured dead ends (TRN2): fp8 DoubleRow projections trip the P0 power
throttle (-20% clocks chip-wide, net loss) and one-sided fp8 q costs ~1.5%
rel err (softmax logit noise passes straight to the output; the 2e-2 gate
leaves no room).  Value-path fp8 (wv/v/e01/wo) alone costs ~3%.
"""

import sys

sys.path.insert(0, "/opt/trn_rl_repo")

import numpy as np
import ml_dtypes

B, S, D, H = 4, 2048, 1024, 16
HD = D // H
SCALE = 1.0 / float(np.sqrt(HD))
NEG = -1e9

# Schraudolph exp-as-int16 constants (bf16 bit layout: 7 mantissa bits)
EXP_A = 128.0 / float(np.log(2.0))      # d bits / d ln(x)
ALPHA = EXP_A * SCALE                   # folded into wq on the host
EXP_B = 128.0 * (127.0 - 0.0573) + 0.5  # bits offset (softmax-invariant)
NEG_DVE = -1.0e6
USE_GP_DMA = True
QP_FP8 = False  # q-side (wq, xq) fp8 DoubleRow: saves ~13us of PE cycles
# but (measured) trips the P0 power throttle -> whole chip -20% clocks,
# net +26us, AND costs ~1.7% rel err (softmax logit noise).  Keep off.
DVE_KCS = (5, 7)  # key chunks whose exp runs as Schraudolph on VectorE.
# Must be late in the step: the previous step's norm chain occupies VectorE
# during kc0-3, and a pair two kc ahead stalls on the s-PSUM buffer until
# the owning exp drains.

DP = 4  # batch shards
TP = 2  # head-group shards
HL = H // TP  # heads per core (8)
DL = HL * HD  # local head dims per core (512)
N_HP = HL // 2  # head pairs per core (4)
QCH = 512  # q chunk (free dim of score matmuls)
KV_P = 128  # key chunk (partition dim of scoresT)
KC8 = D // 128  # contraction chunks for projections (8)

bf16 = ml_dtypes.bfloat16
fp8 = ml_dtypes.float8_e4m3  # TRN FP8_EXP4 (max +-240)


def _windows(n, w=512):
    out = []
    off = 0
    while off < n:
        out.append((off, min(w, n - off)))
        off += w
    return out


def _build(nkv, with_bias=True):
    from concourse import bacc
    import concourse.mybir as mybir
    from concourse.tile import TileContext

    dt = mybir.dt
    f32 = dt.float32
    b16 = dt.bfloat16
    i16 = dt.int16
    qdt = dt.float8e4 if QP_FP8 else b16
    EXP = mybir.ActivationFunctionType.Exp
    ADD = mybir.AluOpType.add
    MAX = mybir.AluOpType.max
    DR = mybir.MatmulPerfMode.DoubleRow

    nkc = nkv // KV_P  # key chunks
    kwins = _windows(nkv)  # kt projection windows
    W1 = KC8 * 128  # weight cols per hp block

    nc = bacc.Bacc(trn_type="TRN2")

    xq_d = nc.dram_tensor("xq", (128, KC8 * S), qdt, kind="ExternalInput").ap()
    xkv_d = nc.dram_tensor("xkv", (128, KC8 * nkv), b16, kind="ExternalInput").ap()
    wq_d = nc.dram_tensor("wqt", (128, N_HP * W1), qdt, kind="ExternalInput").ap()
    wk_d = nc.dram_tensor("wkt", (128, N_HP * W1), b16, kind="ExternalInput").ap()
    wv_d = nc.dram_tensor("wvt", (128, KC8 * DL), b16, kind="ExternalInput").ap()
    if with_bias:
        bq_d = nc.dram_tensor("bq", (1, DL), b16, kind="ExternalInput").ap()
        bk_d = nc.dram_tensor("bk", (1, DL), b16, kind="ExternalInput").ap()
        bv_d = nc.dram_tensor("bv", (1, DL), b16, kind="ExternalInput").ap()
    wo_d = nc.dram_tensor("wot", (128, (DL // 128) * D), b16, kind="ExternalInput").ap()
    mba_d = nc.dram_tensor("mbact", (nkv,), f32, kind="ExternalInput").ap()
    mbd_d = nc.dram_tensor("mbdve", (nkv,), f32, kind="ExternalInput").ap()
    out_d = nc.dram_tensor("out", (S, D), b16, kind="ExternalOutput").ap()

    with TileContext(nc) as tc:
        with (
            tc.tile_pool(name="persist", bufs=1) as pp,
            tc.tile_pool(name="ps_s", bufs=2, space="PSUM") as ps_s,
            tc.tile_pool(name="ps_cc", bufs=1, space="PSUM") as ps_cc,
            tc.tile_pool(name="ps_aux", bufs=2, space="PSUM") as ps_aux,
            tc.tile_pool(name="etile", bufs=12) as ep,
            tc.tile_pool(name="work", bufs=2) as wp,
            tc.tile_pool(name="ob", bufs=3) as obp,
        ):
            # ---- persistent SBUF tensors ----
            xq_sb = pp.tile([128, KC8, S], qdt, tag="xq")
            xq1_sb = pp.tile([1, S], b16, tag="xq1")
            xkv_sb = pp.tile([128, KC8, nkv], b16, tag="xkv")
            xkv1_sb = pp.tile([1, nkv], b16, tag="xkv1")
            wq_sb = pp.tile([128, N_HP, KC8, 128], qdt, tag="wq")
            wk_sb = pp.tile([128, N_HP, KC8, 128], b16, tag="wk")
            wv_sb = pp.tile([128, KC8, DL], b16, tag="wv")
            wq1_sb = pp.tile([1, DL], b16, tag="wq1")
            wk1_sb = pp.tile([1, DL], b16, tag="wk1")
            wv1_sb = pp.tile([1, DL], b16, tag="wv1")
            wo_sb = pp.tile([128, DL // 128, D], b16, tag="wo")
            mba_sb = pp.tile([128, nkc], f32, tag="mba")
            mbd_sb = pp.tile([128, nkc], f32, tag="mbd")
            qt_sb = pp.tile([128, N_HP, S], b16, tag="qt")
            kt_sb = pp.tile([128, N_HP, nkv], b16, tag="kt")
            v_sb = pp.tile([128, nkc, HL, 128], b16, tag="v")
            ctx_sb = pp.tile([128, N_HP, S], b16, tag="ctx")

            # ---- DMA staging, split across the Sync and GpSimd issue
            # queues; ordered so the first score matmuls gate on ~2.5MB ----
            gp = nc.gpsimd if USE_GP_DMA else nc.sync

            def load_w(eng, dst, src, hp):
                eng.dma_start(
                    dst[:, hp].rearrange("p kc e -> p (kc e)"),
                    src[:, hp * W1 : hp * W1 + W1],
                )

            def load_x_win(eng, dst, src, tot, off, n, group=1):
                """Load x window [*, all kc, off:off+n] in ceil(8/group) DMAs.

                group>1 packs several kc chunks into one strided-AP DMA:
                fewer issue slots (each dma_start costs ~0.6us of queue
                time) at the price of fewer DMA engines per window."""
                srcv = src.rearrange("p (kc s) -> p kc s", kc=KC8)
                for kc0 in range(0, KC8, group):
                    eng.dma_start(
                        dst[:, kc0 : kc0 + group, off : off + n],
                        srcv[:, kc0 : kc0 + group, off : off + n],
                    )

            # gating loads are fine-grained and interleaved kc-wise so each
            # early projection matmul only waits for its own kc slices
            # (~0.3MB), not the whole 2.5MB prologue set.  (Consolidating
            # the prologue into few big DMAs was tried -- fewer 0.6us issue
            # slots -- but it makes the supply wave land later and more
            # variably: measured worse.)
            wq_v = wq_d.rearrange("p (h kc e) -> p h kc e", h=N_HP, kc=KC8)
            wk_v = wk_d.rearrange("p (h kc e) -> p h kc e", h=N_HP, kc=KC8)
            xq_v = xq_d.rearrange("p (kc s) -> p kc s", kc=KC8)
            xkv_v = xkv_d.rearrange("p (kc s) -> p kc s", kc=KC8)
            nkv0 = min(512, nkv)
            nc.sync.dma_start(wq_sb[:, 0, 0:2], wq_v[:, 0, 0:2])
            gp.dma_start(wk_sb[:, 0, 0:2], wk_v[:, 0, 0:2])
            for kc0 in range(0, KC8, 2):
                nc.sync.dma_start(
                    xq_sb[:, kc0 : kc0 + 2, 0:512], xq_v[:, kc0 : kc0 + 2, 0:512]
                )
                gp.dma_start(
                    xkv_sb[:, kc0 : kc0 + 2, 0:nkv0],
                    xkv_v[:, kc0 : kc0 + 2, 0:nkv0],
                )
                if kc0 + 2 < KC8:
                    nc.sync.dma_start(
                        wq_sb[:, 0, kc0 + 2 : kc0 + 4], wq_v[:, 0, kc0 + 2 : kc0 + 4]
                    )
                    gp.dma_start(
                        wk_sb[:, 0, kc0 + 2 : kc0 + 4], wk_v[:, 0, kc0 + 2 : kc0 + 4]
                    )
            # ScalarE is a hardware DGE queue too but its sequencer is the
            # busiest (it also dispatches the exps): issue only the small
            # mask biases and xkv w1 (few, grouped) from it.
            nc.scalar.dma_start(mba_sb[:], mba_d.rearrange("(kc p) -> p kc", p=128))
            nc.scalar.dma_start(mbd_sb[:], mbd_d.rearrange("(kc p) -> p kc", p=128))
            if len(kwins) > 1:
                load_x_win(nc.scalar, xkv_sb, xkv_d, nkv, *kwins[1], group=2)
            if with_bias:
                gp.dma_start(wq1_sb[:], bq_d)
                gp.dma_start(wk1_sb[:], bk_d)
                gp.dma_start(wv1_sb[:], bv_d)
            # non-gating loads are grouped (fewer issue slots); ordered by
            # first-use time.  hp weights interleave with the xkv windows
            # they unblock; xq w1 early so the t=0 qwin projection never
            # stalls.
            wv_v = wv_d.rearrange("p (kc e) -> p kc e", kc=KC8)
            for kc0 in range(0, KC8, 4):
                nc.sync.dma_start(
                    wv_sb[:, kc0 : kc0 + 4, :], wv_v[:, kc0 : kc0 + 4, :]
                )
            for hp in range(1, N_HP):
                load_w(gp, wq_sb, wq_d, hp)
                load_w(gp, wk_sb, wk_d, hp)
                if 1 + hp < len(kwins):
                    load_x_win(gp, xkv_sb, xkv_d, nkv, *kwins[1 + hp], group=2)
            xq_wins = _windows(S)[1:]
            load_x_win(nc.sync, xq_sb, xq_d, S, *xq_wins[0], group=2)
            wo_v = wo_d.rearrange("p (kc e) -> p kc e", kc=DL // 128)
            for kc0 in range(0, DL // 128, 2):
                nc.sync.dma_start(
                    wo_sb[:, kc0 : kc0 + 2, :], wo_v[:, kc0 : kc0 + 2, :]
                )
            for off, n in xq_wins[1:]:
                load_x_win(nc.sync, xq_sb, xq_d, S, off, n, group=4)

            # constants
            nc.vector.memset(xq1_sb[:], 1.0)
            if with_bias:
                nc.vector.memset(xkv1_sb[:], 1.0)
            nc.vector.memset(v_sb[:, :, :, 64:128], 1.0)

            # Nominal HAM warm-up (kept from the best-measured config).
            # Note: K=1/M=1 matmuls do NOT register as HAM array activity
            # (measured: clock stays at 1.2GHz), so this is really a ~2us
            # DMA head-start before the first real matmul.  A true
            # full-array warm-up latches 2.4GHz but the early DMA stalls
            # re-throttle it anyway, and its extra delay measured worse.
            wu = ps_aux.tile([128, 512], f32, tag="aux", name="warmup")
            N_WU = 5
            for i in range(N_WU):
                nc.tensor.matmul(
                    wu[0:1, 0:512],
                    lhsT=xq1_sb[:, 0:1],
                    rhs=xq1_sb[:, 0:512],
                    start=(i == 0),
                    stop=(i == N_WU - 1),
                )

            # ================= deferred-unit machinery =================
            # unit = list of chunk thunks (each ~0.4-0.9us of PE work);
            # fill(n) emits n chunks from the queue head; force(key) emits
            # a whole unit immediately (dependency safety).
            units = {}     # key -> list of remaining thunks
            queue = []     # ordered keys
            started = set()  # units with some chunks already emitted (their
            # PSUM aux tile is live; never interleave another unit's chunks
            # before they finish)

            def add_unit(key, thunks, front=False):
                units[key] = list(thunks)
                if front:
                    pos = 1 if (queue and queue[0] in started) else 0
                    queue.insert(pos, key)
                else:
                    queue.append(key)

            def fill(n):
                done = 0
                while done < n and queue:
                    key = queue[0]
                    th = units[key]
                    if th:
                        th.pop(0)()
                        started.add(key)
                        done += 1
                    if not th:
                        queue.pop(0)
                        started.discard(key)
                        del units[key]

            def force(key):
                if key in units:
                    for th in units[key]:
                        th()
                    units[key].clear()
                    if key in queue:
                        queue.remove(key)
                    started.discard(key)
                    del units[key]

            # ---- projection unit builders ----
            def proj_qk_unit(w_sb, w1_sb, dst_sb, hp, off, n, nfree):
                """chunks computing dst[:, hp, off:off+n].  q side (nfree==S)
                runs fp8 DoubleRow (K=256/matmul, 4 instead of 8)."""
                is_q = nfree == S
                x_sb, x1_sb = (xq_sb, xq1_sb) if is_q else (xkv_sb, xkv1_sb)
                dr = QP_FP8 and is_q
                state = {}

                def mm2(kc0):
                    def th():
                        if "ps" not in state:
                            state["ps"] = ps_aux.tile([128, 512], f32, tag="aux", name="auxps")
                        ps = state["ps"]
                        if dr:
                            nc.tensor.matmul(
                                ps[:, :n],
                                lhsT=w_sb[:, hp, kc0 : kc0 + 2, :],
                                rhs=x_sb[:, kc0 : kc0 + 2, off : off + n],
                                start=(kc0 == 0),
                                stop=(not with_bias and kc0 == KC8 - 2),
                                perf_mode=DR,
                            )
                            return
                        for kc in (kc0, kc0 + 1):
                            nc.tensor.matmul(
                                ps[:, :n],
                                lhsT=w_sb[:, hp, kc, :],
                                rhs=x_sb[:, kc, off : off + n],
                                start=(kc == 0),
                                stop=(not with_bias and kc == KC8 - 1),
                            )

                    return th

                def tail():
                    ps = state["ps"]
                    if with_bias:
                        nc.tensor.matmul(
                            ps[:, :n],
                            lhsT=w1_sb[:, hp * 128 : hp * 128 + 128],
                            rhs=x1_sb[:, off : off + n],
                            start=False,
                            stop=True,
                        )
                    nc.scalar.copy(out=dst_sb[:, hp, off : off + n], in_=ps[:, :n])

                return [mm2(0), mm2(2), mm2(4), lambda: (mm2(6)(), tail())]

            def v_unit(mt, half):
                """v[keys mt*128:+128, heads half*4..+4] into v_sb."""
                hs = slice(half * 256, half * 256 + 256)
                state = {}

                def mm4(kc0):
                    def th():
                        if "ps" not in state:
                            state["ps"] = ps_aux.tile([128, 512], f32, tag="aux", name="auxps")
                        ps = state["ps"]
                        for kc in range(kc0, kc0 + 4):
                            nc.tensor.matmul(
                                ps[:, 0:256],
                                lhsT=xkv_sb[:, kc, mt * 128 : mt * 128 + 128],
                                rhs=wv_sb[:, kc, hs],
                                start=(kc == 0),
                                stop=(not with_bias and kc == KC8 - 1),
                            )

                    return th

                def tail():
                    ps = state["ps"]
                    if with_bias:
                        nc.tensor.matmul(
                            ps[:, 0:256],
                            lhsT=xkv1_sb[:, mt * 128 : mt * 128 + 128],
                            rhs=wv1_sb[:, hs],
                            start=False,
                            stop=True,
                        )
                    nc.vector.tensor_copy(
                        out=v_sb[:, mt, half * 4 : half * 4 + 4, 0:64],
                        in_=ps[:, 0:256].rearrange("p (h e) -> p h e", h=4),
                    )

                return [mm4(0), lambda: (mm4(4)(), tail())]

            def op_unit(rt):
                """out-projection for row-tile rt: 2 psum halves + copies."""
                rs = slice(rt * 128, rt * 128 + 128)
                state = {}

                def mm2(nj, k0):
                    def th():
                        key = f"ps{nj}"
                        if key not in state:
                            state[key] = ps_aux.tile([128, 512], f32, tag="aux", name="auxps")
                        ps = state[key]
                        ns = slice(nj * 512, nj * 512 + 512)
                        for khp in (k0, k0 + 1):
                            nc.tensor.matmul(
                                ps[:],
                                lhsT=ctx_sb[:, khp, rs],
                                rhs=wo_sb[:, khp, ns],
                                start=(khp == 0),
                                stop=(khp == N_HP - 1),
                            )

                    return th

                def copy(nj):
                    if "ob" not in state:
                        state["ob"] = obp.tile([128, D], b16, tag="ob", name="obt")
                    nc.vector.tensor_copy(
                        out=state["ob"][:, nj * 512 : nj * 512 + 512],
                        in_=state[f"ps{nj}"][:],
                    )

                def tail():
                    mm2(1, 2)()
                    copy(1)
                    nc.sync.dma_start(out_d[rs, :], state["ob"][:])

                # four explicitly-placeable thunks: A-chunks (khp 0,1) are
                # ready immediately; B-chunks (khp 2,3) wait on the previous
                # step's norm, which runs kc0-3 on VectorE -- the step
                # scheduler places them at kc>=5
                return {
                    "a0": mm2(0, 0),
                    "a1": mm2(1, 0),
                    "b0": lambda: (mm2(0, 2)(), copy(0)),
                    "b1": tail,
                }

            # tail out-projection for the last q-chunk, split in two phases:
            # phase 1 (khp 0,1) depends only on earlier steps and keeps the
            # PE warm while the last norm chain runs; phase 2 (khp 2,3 +
            # copy + DMA) waits on the final normalizations.  PSUM comes
            # from the s-pool ([128,1024] tiles, free once exps are done).
            op_tail_state = {}

            def op_tail_p1(rt, pool="s"):
                rs = slice(rt * 128, rt * 128 + 128)

                def th():
                    if pool == "s":
                        ps = ps_s.tile([128, D], f32, tag="s", name="opt")
                        halves = [ps[:, 0:512], ps[:, 512:1024]]
                    else:
                        a = ps_aux.tile([128, 512], f32, tag="aux", name="auxps")
                        b = ps_aux.tile([128, 512], f32, tag="aux", name="auxps")
                        halves = [a[:], b[:]]
                    op_tail_state[rt] = halves
                    for nj in range(D // 512):
                        ns = slice(nj * 512, nj * 512 + 512)
                        for khp in (0, 1):
                            nc.tensor.matmul(
                                halves[nj],
                                lhsT=ctx_sb[:, khp, rs],
                                rhs=wo_sb[:, khp, ns],
                                start=(khp == 0),
                                stop=False,
                            )

                return [th]

            def op_tail_p2a(rt):
                """khp=2 accumulation: depends on the second-to-last norm
                only, so it bridges the final norm chain on the PE."""
                rs = slice(rt * 128, rt * 128 + 128)

                def th():
                    halves = op_tail_state[rt]
                    for nj in range(D // 512):
                        ns = slice(nj * 512, nj * 512 + 512)
                        nc.tensor.matmul(
                            halves[nj],
                            lhsT=ctx_sb[:, 2, rs],
                            rhs=wo_sb[:, 2, ns],
                            start=False,
                            stop=False,
                        )

                return [th]

            def op_tail_p2b(rt):
                rs = slice(rt * 128, rt * 128 + 128)

                def th():
                    halves = op_tail_state[rt]
                    for nj in range(D // 512):
                        ns = slice(nj * 512, nj * 512 + 512)
                        nc.tensor.matmul(
                            halves[nj],
                            lhsT=ctx_sb[:, 3, rs],
                            rhs=wo_sb[:, 3, ns],
                            start=False,
                            stop=True,
                        )
                    # drain-phase: split the two half-copies across ScalarE
                    # and VectorE and DMA each half as soon as it lands, so
                    # the final copy->DMA chain is half as long.
                    ob = obp.tile([128, D], b16, tag="ob", name="obt")
                    nc.scalar.copy(out=ob[:, 0:512], in_=halves[0])
                    nc.vector.tensor_copy(out=ob[:, 512:1024], in_=halves[1])
                    nc.sync.dma_start(out_d[rs, 0:512], ob[:, 0:512])
                    nc.scalar.dma_start(out_d[rs, 512:1024], ob[:, 512:1024])

                return [th]

            def q_key(hp, qc):
                return ("q", hp, qc)

            def k_key(hp, w):
                return ("k", hp, w)

            def v_key(mt, half):
                return ("v", mt, half)

            # prologue: only what gates the first exp
            force_emit = proj_qk_unit(wq_sb, wq1_sb, qt_sb, 0, 0, 512, S)
            for th in force_emit:
                th()
            kw0 = proj_qk_unit(wk_sb, wk1_sb, kt_sb, 0, 0, kwins[0][1], nkv)
            for th in kw0:
                th()

            # queue: rest of kt hp0, then hp1 (needed at t=1), THEN v half0
            # (consumed only by the t=0 drain): v depends on the late-landing
            # wv DMA, and placing it early head-of-line-blocks the PE queue
            # on real hardware when the DMA lands later than the compile-time
            # schedule predicted.  (force() is the dependency-safety net.)
            for wi, (off, n) in enumerate(kwins[1:], start=1):
                add_unit(k_key(0, wi), proj_qk_unit(wk_sb, wk1_sb, kt_sb, 0, off, n, nkv))
            for hp in range(1, N_HP):
                add_unit(q_key(hp, 0), proj_qk_unit(wq_sb, wq1_sb, qt_sb, hp, 0, 512, S))
                for wi, (off, n) in enumerate(kwins):
                    add_unit(
                        k_key(hp, wi),
                        proj_qk_unit(wk_sb, wk1_sb, kt_sb, hp, off, n, nkv),
                    )
                if hp == 1:
                    for mt in range(nkc):
                        add_unit(v_key(mt, 0), v_unit(mt, 0))
                if hp == 2:
                    for mt in range(nkc):
                        add_unit(v_key(mt, 1), v_unit(mt, 1))

            # ================= attention steps =================
            for t in range(4 * N_HP):
                qc, hp = divmod(t, N_HP)
                qs = slice(qc * QCH, qc * QCH + QCH)
                half = hp // 2

                # dependency safety: everything this step reads must be
                # emitted before its consumers.  At t=0 the first ctx
                # matmuls only run in the drain phase, so the v forces wait
                # until then (v gates on the late wv DMA).
                force(q_key(hp, qc))
                for wi in range(len(kwins)):
                    force(k_key(hp, wi))
                if t > 0:
                    for mt in range(nkc):
                        force(v_key(mt, half))

                # out-projection for row-tile (qc-1, hp): all head-pairs of
                # q-chunk qc-1 are complete by now.  Its chunks are placed
                # at explicit kc slots: the khp0,1 halves early, the khp2,3
                # halves (which wait on the previous step's norm chain,
                # running kc0-3 on VectorE) at kc5/kc7.
                slots = {}
                if qc > 0:
                    rt = (qc - 1) * N_HP + hp
                    op = op_unit(rt)
                    slots = {0: op["a0"], 5: op["b0"], 6: op["a1"], 7: op["b1"]}

                cc = ps_cc.tile([128, 2 * QCH], f32, tag="cc")
                c0 = cc[:, 0:QCH]
                c1 = cc[:, QCH : 2 * QCH]

                def ctx_mm(ekc, hp=hp, c0=c0, c1=c1):
                    e01_p, kc_p = ekc
                    nc.tensor.matmul(
                        c0,
                        lhsT=v_sb[:, kc_p, 2 * hp, :],
                        rhs=e01_p[:, 0:QCH],
                        start=(kc_p == 0),
                        stop=(kc_p == nkc - 1),
                    )
                    nc.tensor.matmul(
                        c1,
                        lhsT=v_sb[:, kc_p, 2 * hp + 1, :],
                        rhs=e01_p[:, QCH : 2 * QCH],
                        start=(kc_p == 0),
                        stop=(kc_p == nkc - 1),
                    )

                depth = nkc if t == 0 else 3
                pending = []
                # score pairs for two adjacent kc emitted back-to-back: the
                # PE never overlaps row-group streams, but a K=64 pair that
                # is FOLLOWED by a full-array matmul pays a ~107ns drain
                # penalty -- adjacent pairs amortize it over 2 kc.
                for g0 in range(0, nkc, 2):
                    grp = range(g0, min(g0 + 2, nkc))
                    stiles = []
                    for kc in grp:
                        ks = slice(kc * KV_P, kc * KV_P + KV_P)
                        s01 = ps_s.tile([128, 2 * QCH], f32, tag="s")
                        nc.tensor.matmul(
                            s01[:, 0:QCH],
                            lhsT=kt_sb[0:64, hp, ks],
                            rhs=qt_sb[0:64, hp, qs],
                        )
                        nc.tensor.matmul(
                            s01[:, QCH : 2 * QCH],
                            lhsT=kt_sb[64:128, hp, ks],
                            rhs=qt_sb[64:128, hp, qs],
                        )
                        stiles.append((kc, s01))
                    for kc, s01 in stiles:
                        e01 = ep.tile([128, 2 * QCH], b16, tag="e")
                        if kc in DVE_KCS and t < 4 * N_HP - 1:
                            # Schraudolph exp on VectorE (bf16 bits, int16)
                            nc.vector.tensor_scalar(
                                e01[:].bitcast(i16),
                                s01[:],
                                mbd_sb[:, kc : kc + 1],
                                0.0,
                                ADD,
                                MAX,
                            )
                        else:
                            # exact exp on ScalarE (scores pre-scaled)
                            nc.scalar.activation(
                                e01[:],
                                s01[:],
                                EXP,
                                bias=mba_sb[:, kc : kc + 1],
                                scale=1.0 / EXP_A,
                            )
                        if kc in slots:
                            slots[kc]()
                        else:
                            fill(1)
                        pending.append((e01, kc))
                        if len(pending) > depth:
                            ctx_mm(pending.pop(0))
                if t == 0:
                    for mt in range(nkc):
                        force(v_key(mt, half))
                for p in pending:
                    fill(2)
                    ctx_mm(p)

                # normalize: rows 64-127 of cc hold both heads' denominators
                # (replicated); relocate to base partition 0 (fast-reciprocal
                # breaks on shifted/PSUM APs), one reciprocal, two multiplies.
                # Deferred into the next step's kc loop (see above) so the
                # serial chain doesn't head-of-line-block the DVE exps.
                def norm(hp=hp, qs=qs, cc=cc, c0=c0, c1=c1):
                    den01 = wp.tile([64, 2 * QCH], f32, tag="den", name="den")
                    nc.vector.tensor_copy(out=den01[:], in_=cc[64:128, :])
                    rc01 = wp.tile([64, 2 * QCH], f32, tag="rc", name="rc")
                    nc.vector.reciprocal_approx_fast(rc01[:], den01[:])
                    nc.vector.tensor_mul(
                        out=ctx_sb[0:64, hp, qs], in0=c0[0:64, :], in1=rc01[:, 0:QCH]
                    )
                    nc.vector.tensor_mul(
                        out=ctx_sb[64:128, hp, qs],
                        in0=c1[0:64, :],
                        in1=rc01[:, QCH : 2 * QCH],
                    )

                # (a query-half-split variant of the last norm -- to unblock
                # the first p2b out-projections earlier -- measured ~2us
                # WORSE overall across two runs; reverted)
                norm()

                # queue the q window needed a full qc ahead
                if qc < 3:
                    add_unit(
                        q_key(hp, qc + 1),
                        proj_qk_unit(
                            wq_sb, wq1_sb, qt_sb, hp, (qc + 1) * 512, 512, S
                        ),
                    )

            # drain: last q-chunk's out-projections, phase-split so the PE
            # stays warm across the two final norm chains: p1 (khp0,1) needs
            # norm(t=13), p2a (khp2) needs norm(t=14), p2b (khp3 + copies +
            # DMA) needs the last norm(t=15)
            order = [
                ("opt1", 12, "s"),
                ("opt1", 13, "s"),
                ("opt1", 14, "aux"),
                ("p2a", 12, None),
                ("opt1", 15, "s"),
                ("p2a", 13, None),
                ("p2a", 14, None),
                ("p2a", 15, None),
                ("p2b", 12, None),
                ("p2b", 13, None),
                ("p2b", 14, None),
                ("p2b", 15, None),
            ]
            builders = {"opt1": op_tail_p1, "p2a": op_tail_p2a, "p2b": op_tail_p2b}
            for kind, rt, pool in order:
                add_unit(
                    (kind, rt),
                    op_tail_p1(rt, pool) if kind == "opt1" else builders[kind](rt),
                )
            while queue:
                fill(1)

    nc.finalize()
    return nc


def _pack(a, kc, dtype=bf16):
    """[kc*128, n] -> [128, kc*n] partition-major (SBUF layout)."""
    k128, n = a.shape
    return (
        np.ascontiguousarray(a.reshape(kc, 128, n).transpose(1, 0, 2))
        .reshape(128, kc * n)
        .astype(dtype)
    )


def _pack_w_hp(wT, dtype=bf16):
    """[D, DL] transposed weight -> [128, N_HP*KC8*128] hp-major."""
    a = wT.reshape(KC8, 128, N_HP, 128).transpose(1, 2, 0, 3)
    return np.ascontiguousarray(a).reshape(128, N_HP * KC8 * 128).astype(dtype)


def _host_prep(x, mask, wq, bq, wk, bk, wv, bv, wo):
    x = np.asarray(x, dtype=np.float32)
    mask = np.asarray(mask)
    idxs = [np.nonzero(mask[b])[0] for b in range(B)]
    nmax = max(1, max(len(i) for i in idxs))
    nkv = min(S, ((nmax + KV_P - 1) // KV_P) * KV_P)
    with_bias = bool(
        np.any(np.asarray(bq)) or np.any(np.asarray(bk)) or np.any(np.asarray(bv))
    )

    in_maps = []
    for c in range(DP * TP):
        b, g = c // TP, c % TP
        sl = slice(g * DL, g * DL + DL)

        idx = idxs[b]
        xg = np.zeros((nkv, D), dtype=np.float32)
        xg[: len(idx)] = x[b][idx]

        mba = np.full((nkv,), NEG, dtype=np.float32)
        mba[: len(idx)] = 0.0
        mbd = np.full((nkv,), NEG_DVE, dtype=np.float32)
        mbd[: len(idx)] = EXP_B

        qdt = fp8 if QP_FP8 else bf16
        im = {
            "xq": _pack(x[b].T, KC8, qdt),
            "xkv": _pack(xg.T, KC8),
            "wqt": _pack_w_hp(
                np.asarray(wq, dtype=np.float32)[sl, :].T * ALPHA, qdt
            ),
            "wkt": _pack_w_hp(np.asarray(wk, dtype=np.float32)[sl, :].T),
            "wvt": _pack(np.asarray(wv)[sl, :].T, KC8),
            "wot": _pack(np.asarray(wo)[:, sl].T, DL // 128),
            "mbact": mba,
            "mbdve": mbd,
        }
        if with_bias:
            im["bq"] = (np.asarray(bq, dtype=np.float32)[None, sl] * ALPHA).astype(bf16)
            im["bk"] = np.asarray(bk)[None, sl].astype(bf16)
            im["bv"] = np.asarray(bv)[None, sl].astype(bf16)
        in_maps.append(im)
    return nkv, with_bias, in_maps


def kernel(x, mask, wq, bq, wk, bk, wv, bv, wo, bo):
    from concourse.bass_utils import run_bass_kernel_spmd

    nkv, with_bias, in_maps = _host_prep(x, mask, wq, bq, wk, bk, wv, bv, wo)
    nc = _build(nkv, with_bias)
    res = run_bass_kernel_spmd(nc, in_maps, core_ids=list(range(DP * TP)))

    out = np.empty((B, S, D), dtype=np.float32)
    bo = np.asarray(bo, dtype=np.float32)
    for b in range(B):
        out[b] = (
            res.results[b * TP]["out"].astype(np.float32)
            + res.results[b * TP + 1]["out"].astype(np.float32)
            + bo
        )
    return out



# revision 73
# speedup vs baseline: 1.0165x; 1.0165x over previous
"""Multi-head attention (B=4, S=2048, D=1024, H=16) on 8 TRN2 NeuronCores.

Sharding: DP=4 over batch x TP=2 over heads. Core c handles batch c//2 and
heads 8*(c%2) .. 8*(c%2)+8. Each core computes a partial output [S, D] (its
heads' contribution to the out-projection); the host sums the two TP partials
per batch and adds the output bias.

Key compaction: the key-padding mask removes ~half the keys, so the host
gathers unmasked key rows per batch (padded to a multiple of 128). k/v
projections and attention only touch NKV ~= S/2 keys; padding keys carry a
big negative additive bias fused into the exp so they contribute exactly 0.

On-chip layouts (all matmul operands bf16, accumulation fp32 in PSUM):
  qT/kT : [hd, seq] with the two heads of a pair stacked on partitions
          (0-63 / 64-127).  The two K=64 score matmuls of a pair are
          row-group-packed but the PE never overlaps their streams; a
          full-array matmul following a pair pays ~107ns of drain.  qT is
          PRE-SCALED by ALPHA = SCALE * 128/ln2 so PSUM scores are in
          "bf16-bits" units.
  exp   : split across two engines so neither paces the kernel. ScalarE
          computes exact exp via its free affine (scale=1/A, bias=mask);
          for kc in DVE_KCS (late in the step: the previous step's norm
          chain occupies VectorE during kc0-3) VectorE computes a
          Schraudolph exp in ONE op: bits16 = max(scores + B, 0) converted
          to int16 and bit-viewed as bf16 (linear-in-log2 approx, ~1.8% rms
          sawtooth, softmax-scale invariant; masked keys -> exactly 0).
  v_aug : [keys, v | ones(64)] -> the ctx matmul accumulates ctxT (rows 0-63)
          and the softmax denominator replicated across rows 64-127, so the
          normalization is a copy + fast-reciprocal + multiply on VectorE
          (fast-reciprocal breaks on shifted/PSUM APs -> the copy stays).

Scheduling: the TileScheduler re-orders instructions by readiness (emission
order is only a hint), so per-engine load balance and dependency slack
matter more than emission adjacency.  The exp-paced kc loop leaves PE slack
that is filled by a global queue of deferred matmul chunks (remaining
projections, v units, next q window, previous step's out-projection);
force() emits any still-queued unit a step depends on before its consumers.
v units are queued late (their wv DMA lands late; early placement
head-of-line-blocks the PE queue).  Input DMAs are spread over the Sync,
GpSimd and Scalar issue queues (a dma_start costs ~0.6us of issue time on
its queue; ScalarE's queue is kept light since it dispatches most exps).
Prologue: gating DMAs are kc-interleaved so the first projection matmuls
gate on ~0.3MB, and garbage micro-matmuls warm the PE clock gate (HAM)
during the DMA window.  Drain: the last out-projections are phase-split
(khp0,1 / khp2 / khp3+copy+DMA) so the PE bridges the final norm chains,
and the final copies/DMAs are split across ScalarE+VectorE and two queues.

Measured dead ends (TRN2): fp8 DoubleRow projections trip the P0 power
throttle (-20% clocks chip-wide, net loss) and one-sided fp8 q costs ~1.5%
rel err (softmax logit noise passes straight to the output; the 2e-2 gate
leaves no room).  Value-path fp8 (wv/v/e01/wo) alone costs ~3%.
"""

import sys

sys.path.insert(0, "/opt/trn_rl_repo")

import numpy as np
import ml_dtypes

B, S, D, H = 4, 2048, 1024, 16
HD = D // H
SCALE = 1.0 / float(np.sqrt(HD))
NEG = -1e9

# Schraudolph exp-as-int16 constants (bf16 bit layout: 7 mantissa bits)
EXP_A = 128.0 / float(np.log(2.0))      # d bits / d ln(x)
ALPHA = EXP_A * SCALE                   # folded into wq on the host
EXP_B = 128.0 * (127.0 - 0.0573) + 0.5  # bits offset (softmax-invariant)
NEG_DVE = -1.0e6
USE_GP_DMA = True
QP_FP8 = False  # q-side (wq, xq) fp8 DoubleRow: saves ~13us of PE cycles
# but (measured) trips the P0 power throttle -> whole chip -20% clocks,
# net +26us, AND costs ~1.7% rel err (softmax logit noise).  Keep off.
DVE_KCS = (5, 7)  # key chunks whose exp runs as Schraudolph on VectorE.
# Must be late in the step: the previous step's norm chain occupies VectorE
# during kc0-3, and a pair two kc ahead stalls on the s-PSUM buffer until
# the owning exp drains.

DP = 4  # batch shards
TP = 2  # head-group shards
HL = H // TP  # heads per core (8)
DL = HL * HD  # local head dims per core (512)
N_HP = HL // 2  # head pairs per core (4)
QCH = 512  # q chunk (free dim of score matmuls)
KV_P = 128  # key chunk (partition dim of scoresT)
KC8 = D // 128  # contraction chunks for projections (8)

bf16 = ml_dtypes.bfloat16
fp8 = ml_dtypes.float8_e4m3  # TRN FP8_EXP4 (max +-240)


def _windows(n, w=512):
    out = []
    off = 0
    while off < n:
        out.append((off, min(w, n - off)))
        off += w
    return out


def _build(nkv, with_bias=True):
    from concourse import bacc
    import concourse.mybir as mybir
    from concourse.tile import TileContext

    dt = mybir.dt
    f32 = dt.float32
    b16 = dt.bfloat16
    i16 = dt.int16
    qdt = dt.float8e4 if QP_FP8 else b16
    EXP = mybir.ActivationFunctionType.Exp
    ADD = mybir.AluOpType.add
    MAX = mybir.AluOpType.max
    DR = mybir.MatmulPerfMode.DoubleRow

    nkc = nkv // KV_P  # key chunks
    kwins = _windows(nkv)  # kt projection windows
    W1 = KC8 * 128  # weight cols per hp block

    nc = bacc.Bacc(trn_type="TRN2")

    xq_d = nc.dram_tensor("xq", (128, KC8 * S), qdt, kind="ExternalInput").ap()
    xkv_d = nc.dram_tensor("xkv", (128, KC8 * nkv), b16, kind="ExternalInput").ap()
    wq_d = nc.dram_tensor("wqt", (128, N_HP * W1), qdt, kind="ExternalInput").ap()
    wk_d = nc.dram_tensor("wkt", (128, N_HP * W1), b16, kind="ExternalInput").ap()
    wv_d = nc.dram_tensor("wvt", (128, KC8 * DL), b16, kind="ExternalInput").ap()
    if with_bias:
        bq_d = nc.dram_tensor("bq", (1, DL), b16, kind="ExternalInput").ap()
        bk_d = nc.dram_tensor("bk", (1, DL), b16, kind="ExternalInput").ap()
        bv_d = nc.dram_tensor("bv", (1, DL), b16, kind="ExternalInput").ap()
    wo_d = nc.dram_tensor("wot", (128, (DL // 128) * D), b16, kind="ExternalInput").ap()
    mba_d = nc.dram_tensor("mbact", (nkv,), f32, kind="ExternalInput").ap()
    mbd_d = nc.dram_tensor("mbdve", (nkv,), f32, kind="ExternalInput").ap()
    out_d = nc.dram_tensor("out", (S, D), b16, kind="ExternalOutput").ap()

    with TileContext(nc) as tc:
        with (
            tc.tile_pool(name="persist", bufs=1) as pp,
            tc.tile_pool(name="ps_s", bufs=2, space="PSUM") as ps_s,
            tc.tile_pool(name="ps_cc", bufs=1, space="PSUM") as ps_cc,
            tc.tile_pool(name="ps_aux", bufs=2, space="PSUM") as ps_aux,
            tc.tile_pool(name="etile", bufs=12) as ep,
            tc.tile_pool(name="work", bufs=2) as wp,
            tc.tile_pool(name="ob", bufs=3) as obp,
        ):
            # ---- persistent SBUF tensors ----
            xq_sb = pp.tile([128, KC8, S], qdt, tag="xq")
            xq1_sb = pp.tile([1, S], b16, tag="xq1")
            xkv_sb = pp.tile([128, KC8, nkv], b16, tag="xkv")
            xkv1_sb = pp.tile([1, nkv], b16, tag="xkv1")
            wq_sb = pp.tile([128, N_HP, KC8, 128], qdt, tag="wq")
            wk_sb = pp.tile([128, N_HP, KC8, 128], b16, tag="wk")
            wv_sb = pp.tile([128, KC8, DL], b16, tag="wv")
            wq1_sb = pp.tile([1, DL], b16, tag="wq1")
            wk1_sb = pp.tile([1, DL], b16, tag="wk1")
            wv1_sb = pp.tile([1, DL], b16, tag="wv1")
            wo_sb = pp.tile([128, DL // 128, D], b16, tag="wo")
            mba_sb = pp.tile([128, nkc], f32, tag="mba")
            mbd_sb = pp.tile([128, nkc], f32, tag="mbd")
            qt_sb = pp.tile([128, N_HP, S], b16, tag="qt")
            kt_sb = pp.tile([128, N_HP, nkv], b16, tag="kt")
            v_sb = pp.tile([128, nkc, HL, 128], b16, tag="v")
            ctx_sb = pp.tile([128, N_HP, S], b16, tag="ctx")

            # ---- DMA staging, split across the Sync and GpSimd issue
            # queues; ordered so the first score matmuls gate on ~2.5MB ----
            gp = nc.gpsimd if USE_GP_DMA else nc.sync

            def load_w(eng, dst, src, hp):
                eng.dma_start(
                    dst[:, hp].rearrange("p kc e -> p (kc e)"),
                    src[:, hp * W1 : hp * W1 + W1],
                )

            def load_x_win(eng, dst, src, tot, off, n, group=1):
                """Load x window [*, all kc, off:off+n] in ceil(8/group) DMAs.

                group>1 packs several kc chunks into one strided-AP DMA:
                fewer issue slots (each dma_start costs ~0.6us of queue
                time) at the price of fewer DMA engines per window."""
                srcv = src.rearrange("p (kc s) -> p kc s", kc=KC8)
                for kc0 in range(0, KC8, group):
                    eng.dma_start(
                        dst[:, kc0 : kc0 + group, off : off + n],
                        srcv[:, kc0 : kc0 + group, off : off + n],
                    )

            # gating loads are fine-grained and interleaved kc-wise so each
            # early projection matmul only waits for its own kc slices
            # (~0.3MB), not the whole 2.5MB prologue set.  (Consolidating
            # the prologue into few big DMAs was tried -- fewer 0.6us issue
            # slots -- but it makes the supply wave land later and more
            # variably: measured worse.)
            wq_v = wq_d.rearrange("p (h kc e) -> p h kc e", h=N_HP, kc=KC8)
            wk_v = wk_d.rearrange("p (h kc e) -> p h kc e", h=N_HP, kc=KC8)
            xq_v = xq_d.rearrange("p (kc s) -> p kc s", kc=KC8)
            xkv_v = xkv_d.rearrange("p (kc s) -> p kc s", kc=KC8)
            nkv0 = min(512, nkv)
            nc.sync.dma_start(wq_sb[:, 0, 0:2], wq_v[:, 0, 0:2])
            gp.dma_start(wk_sb[:, 0, 0:2], wk_v[:, 0, 0:2])
            for kc0 in range(0, KC8, 2):
                nc.sync.dma_start(
                    xq_sb[:, kc0 : kc0 + 2, 0:512], xq_v[:, kc0 : kc0 + 2, 0:512]
                )
                gp.dma_start(
                    xkv_sb[:, kc0 : kc0 + 2, 0:nkv0],
                    xkv_v[:, kc0 : kc0 + 2, 0:nkv0],
                )
                if kc0 + 2 < KC8:
                    nc.sync.dma_start(
                        wq_sb[:, 0, kc0 + 2 : kc0 + 4], wq_v[:, 0, kc0 + 2 : kc0 + 4]
                    )
                    gp.dma_start(
                        wk_sb[:, 0, kc0 + 2 : kc0 + 4], wk_v[:, 0, kc0 + 2 : kc0 + 4]
                    )
            # ScalarE is a hardware DGE queue too but its sequencer is the
            # busiest (it also dispatches the exps): issue only the small
            # mask biases and xkv w1 (few, grouped) from it.
            nc.scalar.dma_start(mba_sb[:], mba_d.rearrange("(kc p) -> p kc", p=128))
            nc.scalar.dma_start(mbd_sb[:], mbd_d.rearrange("(kc p) -> p kc", p=128))
            if len(kwins) > 1:
                load_x_win(nc.scalar, xkv_sb, xkv_d, nkv, *kwins[1], group=2)
            if with_bias:
                gp.dma_start(wq1_sb[:], bq_d)
                gp.dma_start(wk1_sb[:], bk_d)
                gp.dma_start(wv1_sb[:], bv_d)
            # non-gating loads are grouped (fewer issue slots); ordered by
            # first-use time.  hp weights interleave with the xkv windows
            # they unblock; xq w1 early so the t=0 qwin projection never
            # stalls.
            wv_v = wv_d.rearrange("p (kc e) -> p kc e", kc=KC8)
            for kc0 in range(0, KC8, 4):
                nc.sync.dma_start(
                    wv_sb[:, kc0 : kc0 + 4, :], wv_v[:, kc0 : kc0 + 4, :]
                )
            for hp in range(1, N_HP):
                load_w(gp, wq_sb, wq_d, hp)
                load_w(gp, wk_sb, wk_d, hp)
                if 1 + hp < len(kwins):
                    load_x_win(gp, xkv_sb, xkv_d, nkv, *kwins[1 + hp], group=2)
            xq_wins = _windows(S)[1:]
            load_x_win(nc.sync, xq_sb, xq_d, S, *xq_wins[0], group=2)
            wo_v = wo_d.rearrange("p (kc e) -> p kc e", kc=DL // 128)
            for kc0 in range(0, DL // 128, 2):
                nc.sync.dma_start(
                    wo_sb[:, kc0 : kc0 + 2, :], wo_v[:, kc0 : kc0 + 2, :]
                )
            for off, n in xq_wins[1:]:
                load_x_win(nc.sync, xq_sb, xq_d, S, off, n, group=4)

            # constants
            nc.vector.memset(xq1_sb[:], 1.0)
            if with_bias:
                nc.vector.memset(xkv1_sb[:], 1.0)
            nc.vector.memset(v_sb[:, :, :, 64:128], 1.0)

            # Nominal HAM warm-up (kept from the best-measured config).
            # Note: K=1/M=1 matmuls do NOT register as HAM array activity
            # (measured: clock stays at 1.2GHz), so this is really a ~2us
            # DMA head-start before the first real matmul.  A true
            # full-array warm-up latches 2.4GHz but the early DMA stalls
            # re-throttle it anyway, and its extra delay measured worse.
            wu = ps_aux.tile([128, 512], f32, tag="aux", name="warmup")
            N_WU = 5
            for i in range(N_WU):
                nc.tensor.matmul(
                    wu[0:1, 0:512],
                    lhsT=xq1_sb[:, 0:1],
                    rhs=xq1_sb[:, 0:512],
                    start=(i == 0),
                    stop=(i == N_WU - 1),
                )

            # ================= deferred-unit machinery =================
            # unit = list of chunk thunks (each ~0.4-0.9us of PE work);
            # fill(n) emits n chunks from the queue head; force(key) emits
            # a whole unit immediately (dependency safety).
            units = {}     # key -> list of remaining thunks
            queue = []     # ordered keys
            started = set()  # units with some chunks already emitted (their
            # PSUM aux tile is live; never interleave another unit's chunks
            # before they finish)

            def add_unit(key, thunks, front=False):
                units[key] = list(thunks)
                if front:
                    pos = 1 if (queue and queue[0] in started) else 0
                    queue.insert(pos, key)
                else:
                    queue.append(key)

            def fill(n):
                done = 0
                while done < n and queue:
                    key = queue[0]
                    th = units[key]
                    if th:
                        th.pop(0)()
                        started.add(key)
                        done += 1
                    if not th:
                        queue.pop(0)
                        started.discard(key)
                        del units[key]

            def force(key):
                if key in units:
                    for th in units[key]:
                        th()
                    units[key].clear()
                    if key in queue:
                        queue.remove(key)
                    started.discard(key)
                    del units[key]

            # ---- projection unit builders ----
            def proj_qk_unit(w_sb, w1_sb, dst_sb, hp, off, n, nfree):
                """chunks computing dst[:, hp, off:off+n].  q side (nfree==S)
                runs fp8 DoubleRow (K=256/matmul, 4 instead of 8)."""
                is_q = nfree == S
                x_sb, x1_sb = (xq_sb, xq1_sb) if is_q else (xkv_sb, xkv1_sb)
                dr = QP_FP8 and is_q
                state = {}

                def mm2(kc0):
                    def th():
                        if "ps" not in state:
                            state["ps"] = ps_aux.tile([128, 512], f32, tag="aux", name="auxps")
                        ps = state["ps"]
                        if dr:
                            nc.tensor.matmul(
                                ps[:, :n],
                                lhsT=w_sb[:, hp, kc0 : kc0 + 2, :],
                                rhs=x_sb[:, kc0 : kc0 + 2, off : off + n],
                                start=(kc0 == 0),
                                stop=(not with_bias and kc0 == KC8 - 2),
                                perf_mode=DR,
                            )
                            return
                        for kc in (kc0, kc0 + 1):
                            nc.tensor.matmul(
                                ps[:, :n],
                                lhsT=w_sb[:, hp, kc, :],
                                rhs=x_sb[:, kc, off : off + n],
                                start=(kc == 0),
                                stop=(not with_bias and kc == KC8 - 1),
                            )

                    return th

                def tail():
                    ps = state["ps"]
                    if with_bias:
                        nc.tensor.matmul(
                            ps[:, :n],
                            lhsT=w1_sb[:, hp * 128 : hp * 128 + 128],
                            rhs=x1_sb[:, off : off + n],
                            start=False,
                            stop=True,
                        )
                    nc.scalar.copy(out=dst_sb[:, hp, off : off + n], in_=ps[:, :n])

                return [mm2(0), mm2(2), mm2(4), lambda: (mm2(6)(), tail())]

            def v_unit(mt, half):
                """v[keys mt*128:+128, heads half*4..+4] into v_sb."""
                hs = slice(half * 256, half * 256 + 256)
                state = {}

                def mm4(kc0):
                    def th():
                        if "ps" not in state:
                            state["ps"] = ps_aux.tile([128, 512], f32, tag="aux", name="auxps")
                        ps = state["ps"]
                        for kc in range(kc0, kc0 + 4):
                            nc.tensor.matmul(
                                ps[:, 0:256],
                                lhsT=xkv_sb[:, kc, mt * 128 : mt * 128 + 128],
                                rhs=wv_sb[:, kc, hs],
                                start=(kc == 0),
                                stop=(not with_bias and kc == KC8 - 1),
                            )

                    return th

                def tail():
                    ps = state["ps"]
                    if with_bias:
                        nc.tensor.matmul(
                            ps[:, 0:256],
                            lhsT=xkv1_sb[:, mt * 128 : mt * 128 + 128],
                            rhs=wv1_sb[:, hs],
                            start=False,
                            stop=True,
                        )
                    nc.vector.tensor_copy(
                        out=v_sb[:, mt, half * 4 : half * 4 + 4, 0:64],
                        in_=ps[:, 0:256].rearrange("p (h e) -> p h e", h=4),
                    )

                return [mm4(0), lambda: (mm4(4)(), tail())]

            def op_unit(rt):
                """out-projection for row-tile rt: 2 psum halves + copies."""
                rs = slice(rt * 128, rt * 128 + 128)
                state = {}

                def mm2(nj, k0):
                    def th():
                        key = f"ps{nj}"
                        if key not in state:
                            state[key] = ps_aux.tile([128, 512], f32, tag="aux", name="auxps")
                        ps = state[key]
                        ns = slice(nj * 512, nj * 512 + 512)
                        for khp in (k0, k0 + 1):
                            nc.tensor.matmul(
                                ps[:],
                                lhsT=ctx_sb[:, khp, rs],
                                rhs=wo_sb[:, khp, ns],
                                start=(khp == 0),
                                stop=(khp == N_HP - 1),
                            )

                    return th

                def copy(nj):
                    if "ob" not in state:
                        state["ob"] = obp.tile([128, D], b16, tag="ob", name="obt")
                    nc.vector.tensor_copy(
                        out=state["ob"][:, nj * 512 : nj * 512 + 512],
                        in_=state[f"ps{nj}"][:],
                    )

                def tail():
                    mm2(1, 2)()
                    copy(1)
                    nc.sync.dma_start(out_d[rs, :], state["ob"][:])

                # four explicitly-placeable thunks: A-chunks (khp 0,1) are
                # ready immediately; B-chunks (khp 2,3) wait on the previous
                # step's norm, which runs kc0-3 on VectorE -- the step
                # scheduler places them at kc>=5
                return {
                    "a0": mm2(0, 0),
                    "a1": mm2(1, 0),
                    "b0": lambda: (mm2(0, 2)(), copy(0)),
                    "b1": tail,
                }

            # tail out-projection for the last q-chunk, split in two phases:
            # phase 1 (khp 0,1) depends only on earlier steps and keeps the
            # PE warm while the last norm chain runs; phase 2 (khp 2,3 +
            # copy + DMA) waits on the final normalizations.  PSUM comes
            # from the s-pool ([128,1024] tiles, free once exps are done).
            op_tail_state = {}

            def op_tail_p1(rt, pool="s"):
                rs = slice(rt * 128, rt * 128 + 128)

                def th():
                    if pool == "s":
                        ps = ps_s.tile([128, D], f32, tag="s", name="opt")
                        halves = [ps[:, 0:512], ps[:, 512:1024]]
                    else:
                        a = ps_aux.tile([128, 512], f32, tag="aux", name="auxps")
                        b = ps_aux.tile([128, 512], f32, tag="aux", name="auxps")
                        halves = [a[:], b[:]]
                    op_tail_state[rt] = halves
                    for nj in range(D // 512):
                        ns = slice(nj * 512, nj * 512 + 512)
                        for khp in (0, 1):
                            nc.tensor.matmul(
                                halves[nj],
                                lhsT=ctx_sb[:, khp, rs],
                                rhs=wo_sb[:, khp, ns],
                                start=(khp == 0),
                                stop=False,
                            )

                return [th]

            def op_tail_p2a(rt):
                """khp=2 accumulation: depends on the second-to-last norm
                only, so it bridges the final norm chain on the PE."""
                rs = slice(rt * 128, rt * 128 + 128)

                def th():
                    halves = op_tail_state[rt]
                    for nj in range(D // 512):
                        ns = slice(nj * 512, nj * 512 + 512)
                        nc.tensor.matmul(
                            halves[nj],
                            lhsT=ctx_sb[:, 2, rs],
                            rhs=wo_sb[:, 2, ns],
                            start=False,
                            stop=False,
                        )

                return [th]

            def op_tail_p2b(rt):
                rs = slice(rt * 128, rt * 128 + 128)

                def th():
                    halves = op_tail_state[rt]
                    for nj in range(D // 512):
                        ns = slice(nj * 512, nj * 512 + 512)
                        nc.tensor.matmul(
                            halves[nj],
                            lhsT=ctx_sb[:, 3, rs],
                            rhs=wo_sb[:, 3, ns],
                            start=False,
                            stop=True,
                        )
                    # drain-phase: split the two half-copies across ScalarE
                    # and VectorE and DMA each half as soon as it lands, so
                    # the final copy->DMA chain is half as long.
                    ob = obp.tile([128, D], b16, tag="ob", name="obt")
                    nc.scalar.copy(out=ob[:, 0:512], in_=halves[0])
                    nc.vector.tensor_copy(out=ob[:, 512:1024], in_=halves[1])
                    nc.sync.dma_start(out_d[rs, 0:512], ob[:, 0:512])
                    nc.scalar.dma_start(out_d[rs, 512:1024], ob[:, 512:1024])

                return [th]

            def q_key(hp, qc):
                return ("q", hp, qc)

            def k_key(hp, w):
                return ("k", hp, w)

            def v_key(mt, half):
                return ("v", mt, half)

            # prologue: only what gates the first exp
            force_emit = proj_qk_unit(wq_sb, wq1_sb, qt_sb, 0, 0, 512, S)
            for th in force_emit:
                th()
            kw0 = proj_qk_unit(wk_sb, wk1_sb, kt_sb, 0, 0, kwins[0][1], nkv)
            for th in kw0:
                th()

            # queue: rest of kt hp0, then hp1 (needed at t=1), THEN v half0
            # (consumed only by the t=0 drain): v depends on the late-landing
            # wv DMA, and placing it early head-of-line-blocks the PE queue
            # on real hardware when the DMA lands later than the compile-time
            # schedule predicted.  (force() is the dependency-safety net.)
            for wi, (off, n) in enumerate(kwins[1:], start=1):
                add_unit(k_key(0, wi), proj_qk_unit(wk_sb, wk1_sb, kt_sb, 0, off, n, nkv))
            for hp in range(1, N_HP):
                add_unit(q_key(hp, 0), proj_qk_unit(wq_sb, wq1_sb, qt_sb, hp, 0, 512, S))
                for wi, (off, n) in enumerate(kwins):
                    add_unit(
                        k_key(hp, wi),
                        proj_qk_unit(wk_sb, wk1_sb, kt_sb, hp, off, n, nkv),
                    )
                if hp == 1:
                    for mt in range(nkc):
                        add_unit(v_key(mt, 0), v_unit(mt, 0))
                if hp == 2:
                    for mt in range(nkc):
                        add_unit(v_key(mt, 1), v_unit(mt, 1))

            # ================= attention steps =================
            for t in range(4 * N_HP):
                qc, hp = divmod(t, N_HP)
                qs = slice(qc * QCH, qc * QCH + QCH)
                half = hp // 2

                # dependency safety: everything this step reads must be
                # emitted before its consumers.  At t=0 the first ctx
                # matmuls only run in the drain phase, so the v forces wait
                # until then (v gates on the late wv DMA).
                force(q_key(hp, qc))
                for wi in range(len(kwins)):
                    force(k_key(hp, wi))
                if t > 0:
                    for mt in range(nkc):
                        force(v_key(mt, half))

                # out-projection for row-tile (qc-1, hp): all head-pairs of
                # q-chunk qc-1 are complete by now.  Its chunks are placed
                # at explicit kc slots: the khp0,1 halves early, the khp2,3
                # halves (which wait on the previous step's norm chain,
                # running kc0-3 on VectorE) at kc5/kc7.
                slots = {}
                if qc > 0:
                    rt = (qc - 1) * N_HP + hp
                    op = op_unit(rt)
                    slots = {0: op["a0"], 5: op["b0"], 6: op["a1"], 7: op["b1"]}

                cc = ps_cc.tile([128, 2 * QCH], f32, tag="cc")
                c0 = cc[:, 0:QCH]
                c1 = cc[:, QCH : 2 * QCH]

                def ctx_mm(ekc, hp=hp, c0=c0, c1=c1):
                    e01_p, kc_p = ekc
                    nc.tensor.matmul(
                        c0,
                        lhsT=v_sb[:, kc_p, 2 * hp, :],
                        rhs=e01_p[:, 0:QCH],
                        start=(kc_p == 0),
                        stop=(kc_p == nkc - 1),
                    )
                    nc.tensor.matmul(
                        c1,
                        lhsT=v_sb[:, kc_p, 2 * hp + 1, :],
                        rhs=e01_p[:, QCH : 2 * QCH],
                        start=(kc_p == 0),
                        stop=(kc_p == nkc - 1),
                    )

                depth = nkc if t == 0 else 4
                pending = []
                # score pairs for two adjacent kc emitted back-to-back: the
                # PE never overlaps row-group streams, but a K=64 pair that
                # is FOLLOWED by a full-array matmul pays a ~107ns drain
                # penalty -- adjacent pairs amortize it over 2 kc.
                for g0 in range(0, nkc, 2):
                    grp = range(g0, min(g0 + 2, nkc))
                    stiles = []
                    for kc in grp:
                        ks = slice(kc * KV_P, kc * KV_P + KV_P)
                        s01 = ps_s.tile([128, 2 * QCH], f32, tag="s")
                        nc.tensor.matmul(
                            s01[:, 0:QCH],
                            lhsT=kt_sb[0:64, hp, ks],
                            rhs=qt_sb[0:64, hp, qs],
                        )
                        nc.tensor.matmul(
                            s01[:, QCH : 2 * QCH],
                            lhsT=kt_sb[64:128, hp, ks],
                            rhs=qt_sb[64:128, hp, qs],
                        )
                        stiles.append((kc, s01))
                    for kc, s01 in stiles:
                        e01 = ep.tile([128, 2 * QCH], b16, tag="e")
                        if kc in DVE_KCS and t < 4 * N_HP - 1:
                            # Schraudolph exp on VectorE (bf16 bits, int16)
                            nc.vector.tensor_scalar(
                                e01[:].bitcast(i16),
                                s01[:],
                                mbd_sb[:, kc : kc + 1],
                                0.0,
                                ADD,
                                MAX,
                            )
                        else:
                            # exact exp on ScalarE (scores pre-scaled)
                            nc.scalar.activation(
                                e01[:],
                                s01[:],
                                EXP,
                                bias=mba_sb[:, kc : kc + 1],
                                scale=1.0 / EXP_A,
                            )
                        if kc in slots:
                            slots[kc]()
                        else:
                            fill(1)
                        pending.append((e01, kc))
                        if len(pending) > depth:
                            ctx_mm(pending.pop(0))
                if t == 0:
                    for mt in range(nkc):
                        force(v_key(mt, half))
                for p in pending:
                    fill(2)
                    ctx_mm(p)

                # normalize: rows 64-127 of cc hold both heads' denominators
                # (replicated); relocate to base partition 0 (fast-reciprocal
                # breaks on shifted/PSUM APs), one reciprocal, two multiplies.
                # Deferred into the next step's kc loop (see above) so the
                # serial chain doesn't head-of-line-block the DVE exps.
                def norm(hp=hp, qs=qs, cc=cc, c0=c0, c1=c1):
                    den01 = wp.tile([64, 2 * QCH], f32, tag="den", name="den")
                    nc.vector.tensor_copy(out=den01[:], in_=cc[64:128, :])
                    rc01 = wp.tile([64, 2 * QCH], f32, tag="rc", name="rc")
                    nc.vector.reciprocal_approx_fast(rc01[:], den01[:])
                    nc.vector.tensor_mul(
                        out=ctx_sb[0:64, hp, qs], in0=c0[0:64, :], in1=rc01[:, 0:QCH]
                    )
                    nc.vector.tensor_mul(
                        out=ctx_sb[64:128, hp, qs],
                        in0=c1[0:64, :],
                        in1=rc01[:, QCH : 2 * QCH],
                    )

                # (a query-half-split variant of the last norm -- to unblock
                # the first p2b out-projections earlier -- measured ~2us
                # WORSE overall across two runs; reverted)
                norm()

                # queue the q window needed a full qc ahead
                if qc < 3:
                    add_unit(
                        q_key(hp, qc + 1),
                        proj_qk_unit(
                            wq_sb, wq1_sb, qt_sb, hp, (qc + 1) * 512, 512, S
                        ),
                    )

            # drain: last q-chunk's out-projections, phase-split so the PE
            # stays warm across the two final norm chains: p1 (khp0,1) needs
            # norm(t=13), p2a (khp2) needs norm(t=14), p2b (khp3 + copies +
            # DMA) needs the last norm(t=15)
            order = [
                ("opt1", 12, "s"),
                ("opt1", 13, "s"),
                ("opt1", 14, "aux"),
                ("p2a", 12, None),
                ("opt1", 15, "s"),
                ("p2a", 13, None),
                ("p2a", 14, None),
                ("p2a", 15, None),
                ("p2b", 12, None),
                ("p2b", 13, None),
                ("p2b", 14, None),
                ("p2b", 15, None),
            ]
            builders = {"opt1": op_tail_p1, "p2a": op_tail_p2a, "p2b": op_tail_p2b}
            for kind, rt, pool in order:
                add_unit(
                    (kind, rt),
                    op_tail_p1(rt, pool) if kind == "opt1" else builders[kind](rt),
                )
            while queue:
                fill(1)

    nc.finalize()
    return nc


def _pack(a, kc, dtype=bf16):
    """[kc*128, n] -> [128, kc*n] partition-major (SBUF layout)."""
    k128, n = a.shape
    return (
        np.ascontiguousarray(a.reshape(kc, 128, n).transpose(1, 0, 2))
        .reshape(128, kc * n)
        .astype(dtype)
    )


def _pack_w_hp(wT, dtype=bf16):
    """[D, DL] transposed weight -> [128, N_HP*KC8*128] hp-major."""
    a = wT.reshape(KC8, 128, N_HP, 128).transpose(1, 2, 0, 3)
    return np.ascontiguousarray(a).reshape(128, N_HP * KC8 * 128).astype(dtype)


def _host_prep(x, mask, wq, bq, wk, bk, wv, bv, wo):
    x = np.asarray(x, dtype=np.float32)
    mask = np.asarray(mask)
    idxs = [np.nonzero(mask[b])[0] for b in range(B)]
    nmax = max(1, max(len(i) for i in idxs))
    nkv = min(S, ((nmax + KV_P - 1) // KV_P) * KV_P)
    with_bias = bool(
        np.any(np.asarray(bq)) or np.any(np.asarray(bk)) or np.any(np.asarray(bv))
    )

    in_maps = []
    for c in range(DP * TP):
        b, g = c // TP, c % TP
        sl = slice(g * DL, g * DL + DL)

        idx = idxs[b]
        xg = np.zeros((nkv, D), dtype=np.float32)
        xg[: len(idx)] = x[b][idx]

        mba = np.full((nkv,), NEG, dtype=np.float32)
        mba[: len(idx)] = 0.0
        mbd = np.full((nkv,), NEG_DVE, dtype=np.float32)
        mbd[: len(idx)] = EXP_B

        qdt = fp8 if QP_FP8 else bf16
        im = {
            "xq": _pack(x[b].T, KC8, qdt),
            "xkv": _pack(xg.T, KC8),
            "wqt": _pack_w_hp(
                np.asarray(wq, dtype=np.float32)[sl, :].T * ALPHA, qdt
            ),
            "wkt": _pack_w_hp(np.asarray(wk, dtype=np.float32)[sl, :].T),
            "wvt": _pack(np.asarray(wv)[sl, :].T, KC8),
            "wot": _pack(np.asarray(wo)[:, sl].T, DL // 128),
            "mbact": mba,
            "mbdve": mbd,
        }
        if with_bias:
            im["bq"] = (np.asarray(bq, dtype=np.float32)[None, sl] * ALPHA).astype(bf16)
            im["bk"] = np.asarray(bk)[None, sl].astype(bf16)
            im["bv"] = np.asarray(bv)[None, sl].astype(bf16)
        in_maps.append(im)
    return nkv, with_bias, in_maps


def kernel(x, mask, wq, bq, wk, bk, wv, bv, wo, bo):
    from concourse.bass_utils import run_bass_kernel_spmd

    nkv, with_bias, in_maps = _host_prep(x, mask, wq, bq, wk, bk, wv, bv, wo)
    nc = _build(nkv, with_bias)
    res = run_bass_kernel_spmd(nc, in_maps, core_ids=list(range(DP * TP)))

    out = np.empty((B, S, D), dtype=np.float32)
    bo = np.asarray(bo, dtype=np.float32)
    for b in range(B):
        out[b] = (
            res.results[b * TP]["out"].astype(np.float32)
            + res.results[b * TP + 1]["out"].astype(np.float32)
            + bo
        )
    return out

